# revision 3
# baseline (speedup 1.0000x reference)
"""DMPNN layer kernel for Trainium2, data-parallel over batch on 8 NeuronCores.

Math (reference):
    gate[i,j]  = (sum_b adj[b,i,j]) > 0                      [N,N], shared across batch
    hW[b,i,o]  = sum_c h[b,i,c] * Wh[o,c]                    Wh = W_w[:, :H]
    term_h     = sum_i gate[i,j] * hW[b,i,o]
    e_sum      = sum_i gate[i,j] * edge_attr[b,i,j,e]
    term_e     = sum_e e_sum[b,j,e] * We[o,e]                We = W_w[:, H:]
    count[j]   = sum_i gate[i,j]
    msg        = term_h + term_e + count[j]*W_b[o]
    msg       *= (j < num_nodes[b])
    h_new      = (h + msg) @ U_w.T + U_b

v2 design (bf16, memory-roofline targeted):
  - All big tensors cast to bf16 on host (rel-err budget 2e-2; measured 4.5e-3).
  - edge_attr host-permuted to [B, N_i, E, N_j] so per-attribute j-slices are
    contiguous: es reduce matmuls get unit-stride rhs, and the gate broadcast
    tile is built with log2 doubling copies instead of 16 strided copies.
  - Weights (Wh^T, We^T, Uw^T) and h^T pre-transposed on host: no on-device
    PE transposes, no identity matrix.
  - gate computed on device from bit-packed adj (int32 word per (i,j), bit b
    = adj[b,i,j]!=0); single compare per element. Broadcast over the E axis
    by doubling copies on GpSimd (1-input ops are ~line rate there).
  - Per batch: 2 ea tiles [128, 4096] bf16 DMA'd (1 MB each), gated in-place
    on DVE (bf16 2x mode), reduced over i by 16 select-matmuls per tile into
    es_ps [16, 256].  msgT [o, j] accumulated in one PSUM group: 2 matmuls
    (term_h) + bias outer product + single k=16 term_e matmul.
  - xT = msg*mask + hT; h_new = xT_chunk.T @ UwT + Ub, written as bf16 and
    upcast on host.
"""

import os
import sys

for _p in ("/opt/trn_rl_repo", "/root/.axon_site/_ro/trn_rl_repo"):
    if _p not in sys.path:
        sys.path.insert(0, _p)

import numpy as np
import ml_dtypes

import concourse.bass as bass
import concourse.tile as tile
from concourse import bacc, mybir
from concourse.bass_utils import run_bass_kernel_spmd

B, N, H, E = 32, 256, 128, 16
N_CORES = 8
BL = B // N_CORES          # batches per core
NJE = N * E                # 4096
F32 = mybir.dt.float32
BF16 = mybir.dt.bfloat16
BF = ml_dtypes.bfloat16


def build_nc(reps: int = 1):
    nc = bacc.Bacc("TRN2", target_bir_lowering=False, debug=False,
                   num_devices=N_CORES)

    d_ea = nc.dram_tensor("ea", [BL, N, E, N], BF16, kind="ExternalInput")
    d_ht = nc.dram_tensor("ht", [BL, H, N], BF16, kind="ExternalInput")
    # adj bit-packed host-side (lossless encoding): bit b of word [i, j] is
    # adj[b, i, j] != 0. The any-over-batch reduction happens on device as
    # a single word != 0 compare per element.
    d_adj = nc.dram_tensor("adjb", [N, N], mybir.dt.int32,
                           kind="ExternalInput")
    d_mask = nc.dram_tensor("mask", [BL, N], BF16, kind="ExternalInput")
    d_wht = nc.dram_tensor("wht", [H, H], BF16, kind="ExternalInput")
    d_wet = nc.dram_tensor("wet", [E, H], BF16, kind="ExternalInput")
    d_uwt = nc.dram_tensor("uwt", [H, H], BF16, kind="ExternalInput")
    d_wb = nc.dram_tensor("wb", [1, H], BF16, kind="ExternalInput")
    d_ub2 = nc.dram_tensor("ub2", [128, 2 * H], BF16, kind="ExternalInput")
    d_ones = nc.dram_tensor("ones", [128, 1], BF16, kind="ExternalInput")
    d_sel16 = nc.dram_tensor("sel16", [128, 256], BF16, kind="ExternalInput")
    d_y = nc.dram_tensor("y", [BL, N, H], BF16, kind="ExternalOutput")

    with tile.TileContext(nc) as tc:
        with (
            tc.tile_pool(name="const", bufs=1) as cpool,
            tc.tile_pool(name="gatep", bufs=1) as gpool,
            tc.tile_pool(name="ea", bufs=5) as eapool,
            tc.tile_pool(name="work", bufs=2) as wpool,
            tc.tile_pool(name="ps_es", bufs=2, space="PSUM") as ps_es,
            tc.tile_pool(name="ps_hw", bufs=2, space="PSUM") as ps_hw,
            tc.tile_pool(name="ps_msg", bufs=2, space="PSUM") as ps_msg,
            tc.tile_pool(name="ps_up", bufs=2, space="PSUM") as ps_up,
        ):
            # ---- constants -------------------------------------------------
            ones = cpool.tile([128, 1], BF16)
            nc.sync.dma_start(ones[:], d_ones[:])
            sel16 = cpool.tile([128, 256], BF16)
            nc.sync.dma_start(sel16[:], d_sel16[:])
            whT = cpool.tile([H, H], BF16)
            nc.sync.dma_start(whT[:], d_wht[:])
            weT = cpool.tile([E, H], BF16)
            nc.sync.dma_start(weT[:], d_wet[:])
            uwT = cpool.tile([H, H], BF16)
            nc.sync.dma_start(uwT[:], d_uwt[:])
            wb = cpool.tile([1, H], BF16)
            nc.sync.dma_start(wb[:], d_wb[:])
            ub2 = cpool.tile([128, 2 * H], BF16)
            nc.sync.dma_start(ub2[:], d_ub2[:])

            for rep in range(reps):
                # ---- gate from adj (all 32 batches via packed words) -------
                gate = []      # per i-chunk: [128, N] bf16 0/1
                gate_bc = []   # per i-chunk: [128, N*E] bf16, e-major bcast
                for c in range(2):
                    at = gpool.tile([128, N], mybir.dt.int32,
                                    name=f"adj_t{c}")
                    nc.sync.dma_start(at[:], d_adj[bass.ts(c, 128), :])
                    g = gpool.tile([128, N], BF16, name=f"gate{c}")
                    nc.vector.tensor_scalar(g[:], at[:], 0, None,
                                            mybir.AluOpType.not_equal)
                    gb = gpool.tile([128, NJE], BF16, name=f"gateb{c}")
                    # e-major: gb[:, e*N + j] = g[:, j]; log2 doubling build
                    nc.gpsimd.tensor_copy(gb[:, 0:N], g[:])
                    w = N
                    while w < NJE:
                        nc.gpsimd.tensor_copy(gb[:, w:2 * w], gb[:, 0:w])
                        w *= 2
                    gate.append(g)
                    gate_bc.append(gb)

                # count[j] = sum_i gate[i, j]  (shares the es_ps pool slot)
                cnt_ps = ps_es.tile([1, N], F32, name="es_ps")
                for c in range(2):
                    nc.tensor.matmul(cnt_ps[:], ones[:], gate[c][:],
                                     start=(c == 0), stop=(c == 1))
                cnt = gpool.tile([1, N], BF16, name="cnt_sb")
                nc.scalar.copy(cnt[:], cnt_ps[:])

                for b in range(BL):
                    # ---- hT / mask / hW ------------------------------------
                    hT = wpool.tile([H, N], BF16, name="hT")
                    nc.sync.dma_start(hT[:], d_ht[b])
                    mrow = wpool.tile([1, N], BF16, name="mrow")
                    nc.sync.dma_start(mrow[:], d_mask[b:b + 1, :])
                    maskb = wpool.tile([128, N], BF16, name="maskb")
                    nc.gpsimd.partition_broadcast(maskb[:], mrow[0:1, :])

                    hw_ps = ps_hw.tile([128, 2 * H], F32, name="hw_ps")
                    for c in range(2):
                        nc.tensor.matmul(hw_ps[:, bass.ts(c, H)],
                                         hT[:, bass.ts(c, 128)], whT[:],
                                         start=True, stop=True)
                    hw = wpool.tile([128, 2 * H], BF16, name="hw")
                    nc.scalar.copy(hw[:], hw_ps[:])

                    # ---- gated edge stream + i-reduction -------------------
                    es_ps = ps_es.tile([E, N], F32, name="es_ps")
                    for c in range(2):
                        ea_t = eapool.tile([128, NJE], BF16, name="ea_t")
                        nc.sync.dma_start(
                            ea_t[:],
                            d_ea[b, bass.ts(c, 128), :, :].rearrange(
                                "p e j -> p (e j)"))
                        nc.vector.tensor_tensor(ea_t[:], ea_t[:],
                                                gate_bc[c][:],
                                                mybir.AluOpType.mult)
                        for e in range(E):
                            nc.tensor.matmul(es_ps[:],
                                             sel16[:, bass.ts(e, E)],
                                             ea_t[:, bass.ts(e, N)],
                                             start=(c == 0 and e == 0),
                                             stop=(c == 1 and e == E - 1))
                    esT = wpool.tile([E, N], BF16, name="esT")
                    nc.scalar.copy(esT[:], es_ps[:])

                    # ---- msgT [o, j] accumulation --------------------------
                    msg_ps = ps_msg.tile([H, N], F32, name="msg_ps")
                    for c in range(2):
                        nc.tensor.matmul(msg_ps[:], hw[:, bass.ts(c, H)],
                                         gate[c][:], start=(c == 0),
                                         stop=False)
                    nc.tensor.matmul(msg_ps[:], wb[:], cnt[:],
                                     start=False, stop=False)
                    nc.tensor.matmul(msg_ps[:], weT[:], esT[:],
                                     start=False, stop=True)

                    # ---- mask + add h --------------------------------------
                    xT = wpool.tile([H, N], BF16, name="xT")
                    nc.vector.tensor_tensor(xT[:], msg_ps[:], maskb[:],
                                            mybir.AluOpType.mult)
                    nc.vector.tensor_tensor(xT[:], xT[:], hT[:],
                                            mybir.AluOpType.add)

                    # ---- h_new = xT.T @ uwT + ub ---------------------------
                    up_ps = ps_up.tile([128, 2 * H], F32, name="up_ps")
                    for c in range(2):
                        nc.tensor.matmul(up_ps[:, bass.ts(c, H)],
                                         xT[:, bass.ts(c, 128)], uwT[:],
                                         start=True, stop=True)
                    yt = wpool.tile([128, 2 * H], BF16, name="yt")
                    nc.vector.tensor_tensor(yt[:], up_ps[:], ub2[:],
                                            mybir.AluOpType.add)
                    for c in range(2):
                        nc.sync.dma_start(d_y[b, bass.ts(c, 128), :],
                                          yt[:, bass.ts(c, H)])

    nc.compile()
    return nc


def _host_prep(h, edge_attr, adj, num_nodes):
    # bit-pack adj: word [i, j] has bit b set iff adj[b, i, j] != 0
    adjb4 = np.packbits(np.asarray(adj) != 0, axis=0, bitorder='little')
    adjb = np.ascontiguousarray(adjb4.transpose(1, 2, 0)).view(
        np.uint32)[:, :, 0].astype(np.int32)
    nn = np.asarray(num_nodes).astype(np.int64)
    mask = (np.arange(N)[None, :] < nn[:, None]).astype(BF)
    ea = np.ascontiguousarray(
        np.asarray(edge_attr, dtype=np.float32).transpose(0, 1, 3, 2)
    ).astype(BF)                                        # [B, N_i, E, N_j]
    ht = np.ascontiguousarray(
        np.asarray(h, dtype=np.float32).transpose(0, 2, 1)).astype(BF)
    return ht, ea, adjb, mask


def _host_consts(W_w, W_b, U_w, U_b):
    ww = np.asarray(W_w, dtype=np.float32)
    return {
        "wht": np.ascontiguousarray(ww[:, :H].T).astype(BF),
        "wet": np.ascontiguousarray(ww[:, H:].T).astype(BF),
        "uwt": np.ascontiguousarray(np.asarray(U_w, np.float32).T).astype(BF),
        "wb": np.asarray(W_b, np.float32).reshape(1, H).astype(BF),
        "ub2": np.tile(np.asarray(U_b, np.float32).reshape(1, H),
                       (128, 2)).astype(BF),
        "ones": np.ones((128, 1), dtype=BF),
        "sel16": np.tile(np.eye(16, dtype=np.float32).reshape(1, 256),
                         (128, 1)).astype(BF),
    }


def kernel(h, edge_attr, adj, num_nodes, W_w, W_b, U_w, U_b):
    ht, ea, adjb, mask = _host_prep(h, edge_attr, adj, num_nodes)
    consts = _host_consts(W_w, W_b, U_w, U_b)

    nc = build_nc(reps=1)
    in_maps = []
    for core in range(N_CORES):
        sl = slice(core * BL, (core + 1) * BL)
        in_maps.append({
            "ht": ht[sl], "ea": ea[sl], "adjb": adjb, "mask": mask[sl],
            **consts,
        })
    res = run_bass_kernel_spmd(nc, in_maps, list(range(N_CORES)))
    out = np.empty((B, N, H), dtype=np.float32)
    for core in range(N_CORES):
        out[core * BL:(core + 1) * BL] = res.results[core]["y"].astype(
            np.float32)
    return out
